# revision 31
# baseline (speedup 1.0000x reference)
"""Trainium2 Bass kernel for 3-layer hetero-GNN message passing (RGCN-style).

Reference semantics (per layer l):
    agg_ss = segment_sum(hs[ss_src], ss_dst) / max(indeg_ss, 1)
    agg_ds = segment_sum(hd[ds_src], ds_dst) / max(indeg_ds, 1)
    hs     = relu(agg_ss @ W_rel[l,0] + agg_ds @ W_rel[l,1] + hs @ W_loop[l] + bias[l])
(doc features hd never change, so agg_ds is layer-invariant.)

Sharding: destination ("sentence") nodes are bin-packed on the host into
128-node bins balanced by in-degree; 104 bins per core x 8 cores.  Edges are
partitioned by destination bin.  Per-relation weights are replicated.  Each
layer ends with 4 per-quartile AllGathers of the updated node features
(quartile-major halo tables) so the next layer's per-quartile gathers can
start as soon as their quartile has landed.

Perf structure:
  - all tables / gathered data / one-hots / weights are bf16 (fp32 PSUM)
  - per-(superbin, quartile) dma_gathers run on the 4 SWDGE queues with an
    enlarged descriptor-ring carveout so descriptor generation pipelines
    against SDMA drain
  - layer-0 gather+one-hot data is host-materialized and bulk-loaded as
    interleaved 4MB transfers alternating between the two HWDGE rings
  - transposed activations (next layer's self-loop lhsT) come from a second
    mirrored matmul chain (W^T stationary), not a PE transpose round-trip
  - recip (1/in-degree) and the layer-invariant doc-relation aggregation
    stay SBUF-resident; per-superbin batched DMAs keep sequencer dispatch low
"""

import os
import sys
import heapq

import numpy as np

for _p in ("/opt/trn_rl_repo", "/root/.axon_site/_ro/trn_rl_repo"):
    if os.path.isdir(_p) and _p not in sys.path:
        sys.path.insert(0, _p)

import ml_dtypes

BF16 = ml_dtypes.bfloat16

P = 128
H = 128
NQ = 4
W = 8          # bins per superbin


class Cfg:
    def __init__(self, ncores, nbins_core, ns, nd, nlayers, nch_ss):
        self.NCORES = ncores
        self.NBINS = nbins_core              # bins per core
        self.SLOTS_CORE = nbins_core * P
        self.SLOTS_TOTAL = self.SLOTS_CORE * ncores
        self.NS = ns
        self.ND = nd
        self.L = nlayers
        self.NCH_SS = nch_ss
        self.NCH_DS = 0
        self.QSIZE = self.SLOTS_TOTAL // NQ  # rows per quartile table


def pack_bins_q(qvec, cls, nbins, nq=4, cap=P, ncand=6):
    """Quartile-aware LPT: balance per-(bin, src-class) in-edge loads.

    qvec [ns, nq]: node's in-edge count per source class.
    cls  [ns]: node's own class.
    Greedy on max class load, with per-bin class capacity cap//nq."""
    tot = qvec.sum(1)
    order = np.argsort(-tot, kind="stable")
    ccap = cap // nq
    counts = np.zeros(nbins, np.int64)
    ccounts = np.zeros((nbins, nq), np.int64)
    qloads = np.zeros((nbins, nq), np.int64)
    key = np.zeros(nbins, np.int64)
    bins = [[] for _ in range(nbins)]
    heap = [(0, b) for b in range(nbins)]
    heapq.heapify(heap)
    for n in order:
        c = int(cls[n])
        v = qvec[n]
        cands, stash = [], []
        while heap and len(cands) < ncand:
            k, b = heapq.heappop(heap)
            if k != key[b] or counts[b] >= cap:
                continue  # stale or permanently full
            if ccounts[b, c] >= ccap:
                stash.append((k, b))  # full for this class only
                continue
            cands.append(b)
        assert cands, "no bin with free class slot"
        best = min(cands, key=lambda b: (int(np.max(qloads[b] + v)),
                                         int(qloads[b].sum())))
        bins[best].append(int(n))
        counts[best] += 1
        ccounts[best, c] += 1
        qloads[best] += v
        key[best] = int(qloads[best].max())
        for k, b in stash:
            heapq.heappush(heap, (k, b))
        for b in cands:
            if counts[b] < cap:
                heapq.heappush(heap, (int(key[b]), b))
    return bins, qloads


def _edge_meta(dst_slot, src_tslot, nbins_total, nch, qsize):
    """Per-(bin, src-quartile) edge arrays for dma_gather.

    dst_slot: global (bin, lane) index of the edge's destination.
    src_tslot: quartile-major table row of the edge's source
        (q = src_tslot // qsize, local row = src_tslot % qsize).
    Within a (bin, quartile) group edge k lands at partition k%128,
    chunk k//128 (dma_gather layout).
    Returns (idx16, lane, glob):
      idx16 [nbins, nq, 128, nch*8] int16  -- local row id, wrapped
          16-partition layout replicated to 128 partitions; pad 0
      lane  [nbins, nq, 128, nch] float32  -- dst lane in bin; pad -1
      glob  [nbins, nq, nch*128] int64     -- global table row (pads map to
          row 0 of the right quartile)
    """
    q_of_edge = src_tslot // qsize
    src_local = (src_tslot % qsize).astype(np.int16)
    bin_of_edge = dst_slot // P
    lane_of_edge = (dst_slot % P).astype(np.float32)
    group = bin_of_edge * NQ + q_of_edge
    order = np.argsort(group, kind="stable")
    g_sorted = group[order]
    src_sorted = src_local[order]
    lane_sorted = lane_of_edge[order]
    ngroups = nbins_total * NQ
    counts = np.bincount(g_sorted, minlength=ngroups)
    starts = np.concatenate([[0], np.cumsum(counts)[:-1]])
    k_in_g = np.arange(len(order)) - starts[g_sorted]
    assert k_in_g.max(initial=0) < nch * P, (k_in_g.max(initial=0), nch * P)
    pp = (k_in_g % P).astype(np.int64)
    cc = (k_in_g // P).astype(np.int64)

    lane = np.full((ngroups, P, nch), -1.0, np.float32)
    lane[g_sorted, pp, cc] = lane_sorted

    idx_flat = np.zeros((ngroups, nch * P), np.int16)
    idx_flat[g_sorted, k_in_g] = src_sorted
    glob = idx_flat.astype(np.int64).reshape(nbins_total, NQ, nch * P)
    qbase = np.arange(NQ, dtype=np.int64) * qsize
    glob = glob + qbase[None, :, None]
    idx16 = idx_flat.reshape(ngroups, nch * 8, 16).transpose(0, 2, 1)
    idx16 = np.broadcast_to(idx16[:, None, :, :], (ngroups, 8, 16, nch * 8))
    idx16 = idx16.reshape(ngroups, P, nch * 8)
    return (np.ascontiguousarray(idx16.reshape(nbins_total, NQ, P, nch * 8)),
            np.ascontiguousarray(lane.reshape(nbins_total, NQ, P, nch)),
            glob)


def _pack_meta_superbins(idx16, lane, nsb):
    """Pack meta per superbin of W consecutive bins.

    Returns [nsb, P, nq*w*nch//2 (lanes bf16) + nq*w*nch*4 (idx16)] int32."""
    nbins, nq, _, nch8 = idx16.shape
    nch = nch8 // 8
    out = []
    for i in range(nsb):
        b0 = i * W
        lane_sb = lane[b0:b0 + W].transpose(2, 1, 0, 3).reshape(P, nq * W * nch)
        lane_i32 = np.ascontiguousarray(lane_sb.astype(BF16)).view(np.int32)
        idx_sb = idx16[b0:b0 + W].transpose(2, 1, 0, 3).reshape(P, nq * W * nch8)
        idx_i32 = np.ascontiguousarray(idx_sb).view(np.int32)
        out.append(np.concatenate([lane_i32, idx_i32], axis=1))
    return np.ascontiguousarray(np.stack(out, axis=0))


def preprocess(inputs, ncores=8, nbins_core=None):
    """Host-side graph partitioning.  Returns (cfg, per-core input maps,
    slot2node) -- slot2node maps device output rows back to node ids."""
    s_feat = np.asarray(inputs["s_feat"], np.float32)
    doc_feat = np.asarray(inputs["doc_feat"], np.float32)
    W_rel = np.asarray(inputs["W_rel"], np.float32)
    W_loop = np.asarray(inputs["W_loop"], np.float32)
    bias = np.asarray(inputs["bias"], np.float32)
    ss_src = np.asarray(inputs["ss_src"], np.int64)
    ss_dst = np.asarray(inputs["ss_dst"], np.int64)
    ds_src = np.asarray(inputs["ds_src"], np.int64)
    ds_dst = np.asarray(inputs["ds_dst"], np.int64)

    ns, h = s_feat.shape
    nd = doc_feat.shape[0]
    nlayers = W_loop.shape[0]
    assert h == H

    if nbins_core is None:
        nbins_core = int(np.ceil(ns / (ncores * P)))
    nbins_core = (nbins_core + W - 1) // W * W  # whole superbins
    nbins_total = nbins_core * ncores
    slots_core = nbins_core * P
    slots_total = nbins_total * P
    qsize = slots_total // NQ
    assert qsize <= 32767

    cnt_ss = np.bincount(ss_dst, minlength=ns)
    cnt_ds = np.bincount(ds_dst, minlength=ns)
    deg_ss = np.maximum(cnt_ss, 1).astype(np.float32)
    deg_ds = np.maximum(cnt_ds, 1).astype(np.float32)

    # node class (= its source quartile) fixed upfront as node_id % 4 so the
    # packer can balance per-(bin, src-class) edge loads; a class-r node gets
    # a lane in [32r, 32r+32) of its bin (quartile-contiguous lanes)
    cls = (np.arange(ns) % NQ).astype(np.int64)
    qvec = np.zeros((ns, NQ), np.int64)
    np.add.at(qvec, (ss_dst, cls[ss_src]), 1)
    bins, _qloads = pack_bins_q(qvec, cls, nbins_total, nq=NQ)

    slot2node = np.full(slots_total, -1, np.int64)   # (bin, lane) -> node
    node2slot = np.full(ns, -1, np.int64)            # node -> (bin, lane)
    node2tslot = np.full(ns, -1, np.int64)           # node -> quartile table row
    for b, nodes in enumerate(bins):
        nxt = [32 * r for r in range(NQ)]
        for n in nodes:
            r = int(cls[n])
            lane_i = nxt[r]
            nxt[r] += 1
            s = b * P + lane_i
            slot2node[s] = n
            node2slot[n] = s
            node2tslot[n] = r * qsize + b * 32 + (lane_i - 32 * r)
    assert (node2slot >= 0).all()

    ss_dst_slot = node2slot[ss_dst]
    ss_src_tslot = node2tslot[ss_src]
    ds_dst_slot = node2slot[ds_dst]

    grp_ss = np.bincount((ss_dst_slot // P) * NQ + ss_src_tslot // qsize,
                         minlength=nbins_total * NQ).max()
    nch_ss = int(np.ceil(grp_ss / P))

    ss_idx16, ss_lane, ss_glob = _edge_meta(ss_dst_slot, ss_src_tslot,
                                            nbins_total, nch_ss, qsize)
    nsb = nbins_core // W
    ssmeta_sb = []
    for c in range(ncores):
        lob = c * nbins_core
        ssmeta_sb.append(_pack_meta_superbins(
            ss_idx16[lob:lob + nbins_core], ss_lane[lob:lob + nbins_core], nsb))

    # host-precomputed doc->sentence aggregation (layer-invariant), in
    # device-row (bin, lane) order
    aggds = np.zeros((slots_total, H), np.float64)
    np.add.at(aggds, ds_dst_slot, doc_feat[ds_src].astype(np.float64))

    valid = slot2node >= 0
    recip_ss = np.ones(slots_total, np.float32)
    recip_ss[valid] = 1.0 / deg_ss[slot2node[valid]]
    recip_ss = recip_ss.astype(BF16)

    deg_ds_slot = np.ones(slots_total, np.float64)
    deg_ds_slot[valid] = deg_ds[slot2node[valid]]
    aggds = (aggds / deg_ds_slot[:, None]).astype(BF16)

    # features in device-row order (sloc) and quartile-major table order (t0g)
    sloc = np.zeros((slots_total, H), BF16)
    sloc[valid] = s_feat[slot2node[valid]].astype(BF16)
    t0g = np.zeros((slots_total, H), BF16)
    t0g[node2tslot[slot2node[valid]]] = sloc[valid]

    # layer-0 gather buffer: chunk (q,c) of bin b holds t0g rows of its edges
    # in dma_gather layout (edge i -> partition i%128, chunk i//128)
    g0 = t0g[ss_glob.reshape(nbins_total, NQ, nch_ss, P)]  # [nb,nq,nch,P,H]
    g0 = np.ascontiguousarray(
        g0.transpose(0, 3, 1, 2, 4).reshape(nbins_total, P, NQ * nch_ss * H))
    # layer-0 one-hot (edge -> dst lane), matching the device s layout
    s0 = (ss_lane[:, :, :, :, None] ==
          np.arange(P, dtype=np.float32)).astype(BF16)   # [nb, NQ, P, NCH, P]
    s0 = np.ascontiguousarray(
        s0.transpose(0, 2, 1, 3, 4).reshape(nbins_total, P, NQ * nch_ss * P))

    iota = np.broadcast_to(
        np.arange(P, dtype=np.float32)[None, :], (P, P)).astype(BF16)

    cfg = Cfg(ncores, nbins_core, ns, nd, nlayers, nch_ss)

    W_rel_bf = W_rel.astype(BF16)
    W_loop_bf = W_loop.astype(BF16)
    bias_bf = bias.astype(BF16)

    in_maps = []
    for c in range(ncores):
        lo, hi = c * slots_core, (c + 1) * slots_core
        lob, hib = c * nbins_core, (c + 1) * nbins_core
        in_maps.append({
            "sT0": np.ascontiguousarray(sloc[lo:hi].T),
            "g0": g0[lob:hib],
            "s0": s0[lob:hib],
            "ssmeta": ssmeta_sb[c],
            "aggdsT": np.ascontiguousarray(aggds[lo:hi].T),
            "recipss": np.ascontiguousarray(
                np.broadcast_to(recip_ss[lo:hi][None, :], (P, slots_core))),
            "wr": W_rel_bf,
            "wl": W_loop_bf,
            "biast": bias_bf,
            "iotat": np.ascontiguousarray(iota),
        })
    return cfg, in_maps, slot2node


def build_program(cfg):
    import concourse.bacc as bacc
    import concourse.mybir as mybir
    import concourse.tile as tile
    from contextlib import ExitStack

    dt = mybir.dt
    f32 = dt.float32
    bf16 = dt.bfloat16
    i32 = dt.int32
    AF = mybir.ActivationFunctionType
    OP = mybir.AluOpType
    L = cfg.L
    NCH = cfg.NCH_SS
    NSB = cfg.NBINS // W
    NKB = NQ * NCH            # chunks per bin
    NKSB = NQ * W * NCH       # chunks per superbin gather group
    NPB = NKB * P             # free elems per bin of g/s data
    NB32 = cfg.NBINS * 32     # rows per quartile in a core's AG shard

    nc = bacc.Bacc("TRN2", target_bir_lowering=False,
                   num_swdge_queues=4, dynamic_dma_scratch_size=40960)

    sT0 = nc.dram_tensor("sT0", [H, cfg.SLOTS_CORE], bf16, kind="ExternalInput")
    g0d = nc.dram_tensor("g0", [cfg.NBINS, P, NPB], bf16, kind="ExternalInput")
    s0d = nc.dram_tensor("s0", [cfg.NBINS, P, NPB], bf16, kind="ExternalInput")
    assert NKSB % 2 == 0
    ssmeta = nc.dram_tensor("ssmeta", [NSB, P, NKSB // 2 + 4 * NKSB], i32,
                            kind="ExternalInput")
    aggdsT = nc.dram_tensor("aggdsT", [H, cfg.SLOTS_CORE], bf16, kind="ExternalInput")
    recipss = nc.dram_tensor("recipss", [P, cfg.SLOTS_CORE], bf16, kind="ExternalInput")
    wr = nc.dram_tensor("wr", [L, 2, H, H], bf16, kind="ExternalInput")
    wl = nc.dram_tensor("wl", [L, H, H], bf16, kind="ExternalInput")
    biast = nc.dram_tensor("biast", [L, H], bf16, kind="ExternalInput")
    iotat = nc.dram_tensor("iotat", [P, P], bf16, kind="ExternalInput")
    out_ext = nc.dram_tensor("out", [cfg.SLOTS_CORE, H], bf16, kind="ExternalOutput")

    # per-layer: quartile-major AG input shard + 4 gathered quartile tables
    ag_sh = [None]
    tq = [None]
    hsT = [sT0]
    for l in range(1, L):
        ag_sh.append(nc.dram_tensor(f"ags{l}", [cfg.SLOTS_CORE, H], bf16))
        tq.append([nc.dram_tensor(f"hsf{l}q{q}", [cfg.QSIZE, H], bf16,
                                  addr_space="Shared") for q in range(NQ)])
        hsT.append(nc.dram_tensor(f"hsT{l}", [H, cfg.SLOTS_CORE], bf16))

    rg = [list(range(cfg.NCORES))]

    with tile.TileContext(nc) as tc, ExitStack() as ctx:
        consts = ctx.enter_context(tc.tile_pool(name="consts", bufs=1))
        meta_p = ctx.enter_context(tc.tile_pool(name="meta", bufs=3))
        gsb_p = ctx.enter_context(tc.tile_pool(name="gsb", bufs=2))
        s_p = ctx.enter_context(tc.tile_pool(name="onehot", bufs=3))
        sm_p = ctx.enter_context(tc.tile_pool(name="small", bufs=4))
        out_p = ctx.enter_context(tc.tile_pool(name="outs", bufs=2))
        ps_agg = ctx.enter_context(tc.tile_pool(name="pagg", bufs=3, space="PSUM"))
        ps_h = ctx.enter_context(tc.tile_pool(name="ph", bufs=3, space="PSUM"))
        ps_t = ctx.enter_context(tc.tile_pool(name="pht", bufs=2, space="PSUM"))

        w0t, w1t, wlt, bt = [], [], [], []
        for l in range(L):
            t = consts.tile([H, H], bf16, tag=f"w0_{l}")
            nc.sync.dma_start(t[:], wr[l, 0])
            w0t.append(t)
            t = consts.tile([H, H], bf16, tag=f"w1_{l}")
            nc.sync.dma_start(t[:], wr[l, 1])
            w1t.append(t)
            t = consts.tile([H, H], bf16, tag=f"wl_{l}")
            nc.sync.dma_start(t[:], wl[l])
            wlt.append(t)
            t = consts.tile([1, H], bf16, tag=f"b_{l}")
            nc.sync.dma_start(t[:], biast[l : l + 1, :])
            bt.append(t)
        iota_t = consts.tile([P, P], bf16, tag="iota")
        nc.sync.dma_start(iota_t[:], iotat[:])
        ones_t = consts.tile([1, H], bf16, tag="ones")
        nc.gpsimd.memset(ones_t[:], 1.0)
        # layer-invariant residents: 1/deg and doc-relation agg (bf16)
        recip_t = consts.tile([P, cfg.SLOTS_CORE], bf16, tag="recip")
        nc.sync.dma_start(recip_t[:], recipss[:])
        aggds_t = consts.tile([H, cfg.SLOTS_CORE], bf16, tag="aggds")
        nc.scalar.dma_start(aggds_t[:], aggdsT[:])

        for l in range(L):
            last = l == L - 1
            for sb in range(NSB):
                m = meta_p.tile([P, NKSB // 2 + 4 * NKSB], i32, tag="m")
                nc.sync.dma_start(m[:], ssmeta[sb])
                lanes = m[:, :NKSB // 2].bitcast(bf16).rearrange(
                    "p (q w n) -> p q w n", q=NQ, w=W)
                # pre-transposed self-loop inputs for the whole superbin
                hts = sm_p.tile([H, W * P], bf16, tag="hts")
                nc.sync.dma_start(hts[:], hsT[l][:, sb * W * P:(sb + 1) * W * P])
                gsb = None
                if l > 0:
                    gsb = gsb_p.tile([P, NKSB * P], bf16, tag="gsb")
                    for q in range(NQ):
                        idx16 = m[:, NKSB // 2 + q * W * NCH * 4
                                  : NKSB // 2 + (q + 1) * W * NCH * 4].bitcast(dt.int16)
                        out3 = gsb[:, q * W * NCH * P : (q + 1) * W * NCH * P
                                   ].rearrange("p (c j) -> p c j", j=P)
                        nc.gpsimd.dma_gather(
                            out_ap=out3,
                            in_ap=tq[l][q][:],
                            idxs_ap=idx16,
                            num_idxs=W * NCH * P, num_idxs_reg=W * NCH * P,
                            elem_size=H, single_packet=False,
                            queue_num=q)
                h_sb = out_p.tile([P, W * H], bf16, tag="h_sb")
                if not last:
                    hT_sb = out_p.tile([H, W * P], bf16, tag="hT_sb")
                for j in range(W):
                    b = sb * W + j
                    if l == 0:
                        # g on the sync ring, s on the scalar ring
                        g = s_p.tile([P, NPB], bf16, tag="g0t")
                        nc.sync.dma_start(g[:], g0d[b])
                        sl0 = s_p.tile([P, NPB], bf16, tag="s")
                        nc.scalar.dma_start(sl0[:], s0d[b])
                        chunk = lambda k, _g=g: _g[:, k * P : (k + 1) * P]
                        svec = lambda k, _s=sl0: _s[:, k * P : (k + 1) * P]
                    else:
                        chunk = lambda k, _j=j: gsb[
                            :, ((k // NCH) * W * NCH + _j * NCH + (k % NCH)) * P
                            : ((k // NCH) * W * NCH + _j * NCH + (k % NCH)) * P + P]
                        s = s_p.tile([P, NKB * P], bf16, tag="s")
                        lanes4 = lanes[:, :, j, :][:, :, :, None].to_broadcast(
                            (P, NQ, NCH, P))
                        iota4 = iota_t[:, None, None, :].to_broadcast((P, NQ, NCH, P))
                        nc.vector.tensor_tensor(
                            out=s[:].rearrange("p (q n j2) -> p q n j2", q=NQ, n=NCH),
                            in0=lanes4, in1=iota4, op=OP.is_equal)
                        svec = lambda k, _s=s: _s[:, k * P : (k + 1) * P]
                    pagg = ps_agg.tile([H, P], f32, tag="pagg")
                    for k in range(NKB):
                        nc.tensor.matmul(
                            out=pagg[:], lhsT=chunk(k), rhs=svec(k),
                            start=(k == 0), stop=(k == NKB - 1))
                    a = sm_p.tile([H, P], bf16, tag="aggT")
                    nc.vector.tensor_tensor(
                        out=a[:], in0=pagg[:],
                        in1=recip_t[:, b * P : (b + 1) * P], op=OP.mult)

                    ph = ps_h.tile([P, H], f32, tag="ph")
                    nc.tensor.matmul(out=ph[:], lhsT=a[:], rhs=w0t[l][:],
                                     start=True, stop=False)
                    nc.tensor.matmul(out=ph[:],
                                     lhsT=aggds_t[:, b * P : (b + 1) * P],
                                     rhs=w1t[l][:], start=False, stop=False)
                    nc.tensor.matmul(out=ph[:], lhsT=hts[:, j * P : (j + 1) * P],
                                     rhs=wlt[l][:], start=False, stop=False)
                    nc.tensor.matmul(out=ph[:], lhsT=ones_t[:], rhs=bt[l][:],
                                     start=False, stop=True)
                    nc.scalar.activation(h_sb[:, j * H : (j + 1) * H], ph[:], AF.Relu)
                    if not last:
                        # transposed activations via a mirrored matmul chain
                        phT = ps_t.tile([H, P], f32, tag="phT")
                        nc.tensor.matmul(out=phT[:], lhsT=w0t[l][:], rhs=a[:],
                                         start=True, stop=False)
                        nc.tensor.matmul(out=phT[:], lhsT=w1t[l][:],
                                         rhs=aggds_t[:, b * P : (b + 1) * P],
                                         start=False, stop=False)
                        nc.tensor.matmul(out=phT[:], lhsT=wlt[l][:],
                                         rhs=hts[:, j * P : (j + 1) * P],
                                         start=False, stop=False)
                        nc.tensor.matmul(out=phT[:], lhsT=bt[l][:], rhs=ones_t[:],
                                         start=False, stop=True)
                        nc.scalar.activation(hT_sb[:, j * P : (j + 1) * P],
                                             phT[:], AF.Relu)
                # superbin-batched stores
                if last:
                    nc.sync.dma_start(
                        out_ext[sb * W * P:(sb + 1) * W * P, :].rearrange(
                            "(w p) h -> p w h", w=W),
                        h_sb[:].rearrange("p (w h) -> p w h", w=W))
                else:
                    # scatter each bin's rows into the quartile-major AG shard
                    agv = ag_sh[l + 1][:].rearrange(
                        "(q g k) h -> q k g h", q=NQ, g=cfg.NBINS)
                    for q in range(NQ):
                        nc.sync.dma_start(
                            agv[q, :, sb * W:(sb + 1) * W, :],
                            h_sb[q * 32:(q + 1) * 32, :].rearrange(
                                "k (w h) -> k w h", w=W))
                    nc.scalar.dma_start(
                        hsT[l + 1][:, sb * W * P:(sb + 1) * W * P], hT_sb[:])
            if not last:
                for q in range(NQ):
                    nc.gpsimd.collective_compute(
                        "AllGather", mybir.AluOpType.bypass,
                        replica_groups=rg,
                        ins=[ag_sh[l + 1][q * NB32:(q + 1) * NB32, :]],
                        outs=[tq[l + 1][q][:]],
                    )
    nc.compile()
    return nc


_CACHE = {}


def _run(cfg, in_maps, **kwargs):
    from concourse.bass_utils import run_bass_kernel_spmd

    key = (cfg.NCORES, cfg.NBINS, cfg.NCH_SS, cfg.ND, cfg.L)
    if key not in _CACHE:
        _CACHE[key] = build_program(cfg)
    nc = _CACHE[key]
    return run_bass_kernel_spmd(nc, in_maps, list(range(cfg.NCORES)), **kwargs)


def kernel(**inputs) -> np.ndarray:
    cfg, in_maps, slot2node = preprocess(inputs, ncores=8)
    results = _run(cfg, in_maps).results
    ns = inputs["s_feat"].shape[0]
    out = np.zeros((ns, H), np.float32)
    full = np.concatenate(
        [np.asarray(results[c]["out"], np.float32) for c in range(cfg.NCORES)],
        axis=0)
    valid = slot2node >= 0
    out[slot2node[valid]] = full[valid]
    return out
